# revision 1
# baseline (speedup 1.0000x reference)
"""AdaptiveJacobianPrunedViT (ViT-B/16, B=64) on 8 TRN2 NeuronCores.

Data-parallel: batch sharded 8 ways (8 items/core), weights replicated.
Per-layer token-importance mean over (B,H) is AllReduce-summed across cores so
all replicas prune identical token sets.

Device layout: feature-major activations x[d, t] (d on partitions, 6x128;
t = item*197 + n, 1576 tokens/core). All matmuls run with weights as the
stationary operand in natural [d_in, d_out] layout, activations moving.
bf16 matmul inputs, fp32 PSUM/residual/softmax-stats/importance.
LayerNorm affine (g,b) is folded into the following matmul's weights on host;
q is pre-scaled by HD**-0.5.

Attention is head-batched: softmax row-sums, CLS rows and vnorm rows for all
12 heads of an item live in [12, T] tiles (one exp/recip/sqrt per chunk
instead of one per head); v is transposed per-ptile; the mask/AllReduce chain
runs after the MLP so its latency hides under the next layer's GEMMs.
"""
import os
import sys

sys.path.insert(0, '/opt/trn_rl_repo')

import numpy as np
import ml_dtypes

import concourse.bass as bass
import concourse.tile as tile
from concourse import bacc, mybir
from concourse.bass_utils import run_bass_kernel_spmd

F32 = mybir.dt.float32
BF16 = mybir.dt.bfloat16
AX = mybir.AxisListType.X
OP = mybir.AluOpType
AF = mybir.ActivationFunctionType

# model dims
B = 64; IMG = 224; P = 16; G = 14; NPATCH = 196; T = 197
D = 768; H = 12; HD = 64; DEPTH = 12; FF = 3072; NCLS = 1000
GAMMA = 0.01; MIN_TOK = 16; EPS = 1e-6; SCALE = HD ** -0.5

NCORE = 8
BL = B // NCORE            # items per core = 8
TOK = BL * T               # tokens per core = 1576
PT = D // 128              # 6 feature ptiles
QT = 3 * D // 128          # 18 qkv feature ptiles
FT = FF // 128             # 24 ff ptiles
NTC = 4                    # token chunks
TCW = TOK // NTC           # 394 tokens per chunk (2 items)
MB = 80.0                  # additive mask bias magnitude (exp(-78) == 0-ish, in LUT range)

bf16 = ml_dtypes.bfloat16

DEPTH_BUILD = int(os.environ.get('KERNEL_DEPTH', str(DEPTH)))


# ---------------------------------------------------------------- host prep
def prep_weights(ii):
    """Fold LN affines + scale into weights; build device-layout arrays."""
    f32 = lambda a: np.ascontiguousarray(np.asarray(a, np.float32))
    out = {}

    ln1_g = f32(ii['ln1_g']); ln1_b = f32(ii['ln1_b'])
    ln2_g = f32(ii['ln2_g']); ln2_b = f32(ii['ln2_b'])
    qkv_w = f32(ii['qkv_w']); qkv_b = f32(ii['qkv_b'])
    proj_w = f32(ii['proj_w']); proj_b = f32(ii['proj_b'])
    fc1_w = f32(ii['fc1_w']); fc1_b = f32(ii['fc1_b'])
    fc2_w = f32(ii['fc2_w']); fc2_b = f32(ii['fc2_b'])

    wqkv = np.empty((DEPTH, PT, 128, 3 * D), bf16)
    bqkv = np.empty((128, DEPTH, QT), np.float32)
    wproj = np.empty((DEPTH, PT, 128, D), bf16)
    bproj = np.empty((128, DEPTH, PT), np.float32)
    wfc1 = np.empty((DEPTH, PT, 128, FF), bf16)
    bfc1 = np.empty((128, DEPTH, FT), np.float32)
    wfc2 = np.empty((DEPTH, FT, 128, D), bf16)
    bfc2 = np.empty((128, DEPTH, PT), np.float32)
    for l in range(DEPTH):
        w = qkv_w[l] * ln1_g[l][:, None]
        b = ln1_b[l] @ qkv_w[l] + qkv_b[l]
        w[:, :D] *= SCALE
        b[:D] *= SCALE
        wqkv[l] = w.reshape(PT, 128, 3 * D).astype(bf16)
        bqkv[:, l, :] = b.reshape(QT, 128).T
        wproj[l] = proj_w[l].reshape(PT, 128, D).astype(bf16)
        bproj[:, l, :] = proj_b[l].reshape(PT, 128).T
        w1 = fc1_w[l] * ln2_g[l][:, None]
        b1 = ln2_b[l] @ fc1_w[l] + fc1_b[l]
        wfc1[l] = w1.reshape(PT, 128, FF).astype(bf16)
        bfc1[:, l, :] = b1.reshape(FT, 128).T
        wfc2[l] = fc2_w[l].reshape(FT, 128, D).astype(bf16)
        bfc2[:, l, :] = fc2_b[l].reshape(PT, 128).T
    out['wqkv'] = wqkv; out['wproj'] = wproj; out['wfc1'] = wfc1; out['wfc2'] = wfc2
    out['biases'] = np.ascontiguousarray(
        np.concatenate([bqkv, bproj, bfc1, bfc2], axis=2))  # [128, 12, 54]

    # patch embed: [3*16*16, 768] ; pos_cls [128, 6, 198]
    patch_w = f32(ii['patch_w'])
    out['wpatch'] = np.ascontiguousarray(patch_w.reshape(PT, 128, D).astype(bf16))
    pos = f32(ii['pos_embed'])[0]            # [197, 768]
    cls0 = f32(ii['cls_token'])[0, 0] + pos[0]  # [768]
    patch_b = f32(ii['patch_b'])
    pc = np.empty((128, PT, T + 1), np.float32)
    posT = pos.T.reshape(PT, 128, T)         # [6,128,197]
    for j in range(PT):
        pc[:, j, 0] = cls0[j * 128:(j + 1) * 128]
        pc[:, j, 1:197] = posT[j, :, 1:] + patch_b[j * 128:(j + 1) * 128, None]
        pc[:, j, 197] = 0.0
    out['pos_cls'] = pc

    # final norm folded into head
    norm_g = f32(ii['norm_g']); norm_b = f32(ii['norm_b'])
    head_w = f32(ii['head_w']); head_b = f32(ii['head_b'])
    hw = head_w * norm_g[:, None]
    hb = norm_b @ head_w + head_b
    out['whead'] = np.ascontiguousarray(hw.reshape(PT, 128, NCLS).astype(bf16))
    bh = np.zeros((128, 8), np.float32)
    bh.reshape(-1, order='F')[:NCLS] = hb    # col fo holds hb[fo*128 : fo*128+128]
    out['bhead'] = bh

    # constants (bf16): stacked I64 | ones | I128 | rsel (12x one-hot-col
    # blocks for head row-sums) | vsel (6x head-pair select blocks)
    cbf = np.zeros((128, 808), np.float32)
    cbf[:, 0:64] = np.tile(np.eye(64, dtype=np.float32), (2, 1))
    cbf[:, 64] = 1.0
    cbf[:, 80:208] = np.eye(128, dtype=np.float32)
    for h in range(H):
        cbf[:, 208 + 13 * h] = 1.0               # rsel[:, h, h] = 1
    for j in range(PT):
        cbf[0:64, 352 + j * 12 + 2 * j] = 1.0    # vsel: head 2j <- partitions 0:64
        cbf[64:128, 352 + j * 12 + 2 * j + 1] = 1.0
    for pl in range(3):                           # bsel6: row-bcast select blocks
        cbf[2 * pl, 424 + pl * 128:424 + pl * 128 + 64] = 1.0
        cbf[2 * pl + 1, 424 + pl * 128 + 64:424 + (pl + 1) * 128] = 1.0
    out['consts_bf'] = cbf.astype(bf16)
    cf = np.zeros((128, 136), np.float32)
    cf[:, 0:128] = np.eye(128, dtype=np.float32)  # identf for col->row transposes
    cf[:, 128] = 1.0                              # ones column (f32)
    cf[0, 129] = EPS
    out['consts_f'] = cf
    return out


def prep_x_shard(x, core):
    """x [B,3,224,224] -> per-core patches, feature-major bf16 [128, 6, 8*196]."""
    xs = np.asarray(x, np.float32)[core * BL:(core + 1) * BL]
    p = xs.reshape(BL, 3, G, P, G, P).transpose(0, 2, 4, 1, 3, 5).reshape(BL, NPATCH, 3 * P * P)
    pT = p.reshape(BL * NPATCH, 3 * P * P).T        # [768, 1568]
    return np.ascontiguousarray(pT.reshape(PT, 128, BL * NPATCH).transpose(1, 0, 2).astype(bf16))


# ---------------------------------------------------------------- builder
def build(nc):
    dt_in = [
        ('xt', [128, PT, BL * NPATCH], BF16),
        ('wqkv', [DEPTH, PT, 128, 3 * D], BF16),
        ('wproj', [DEPTH, PT, 128, D], BF16),
        ('wfc1', [DEPTH, PT, 128, FF], BF16),
        ('wfc2', [DEPTH, FT, 128, D], BF16),
        ('biases', [128, DEPTH, 54], F32),
        ('wpatch', [PT, 128, D], BF16),
        ('pos_cls', [128, PT, T + 1], F32),
        ('whead', [PT, 128, NCLS], BF16),
        ('bhead', [128, 8], F32),
        ('consts_bf', [128, 808], BF16),
        ('consts_f', [128, 136], F32),
    ]
    tens = {n: nc.dram_tensor(n, s, d, kind="ExternalInput") for n, s, d in dt_in}
    out_t = nc.dram_tensor("out", [BL, NCLS], F32, kind="ExternalOutput")

    dbg_names = []
    if os.environ.get('KERNEL_DEBUG_X'):
        dbgx_t = nc.dram_tensor("dbgx", [128, PT, TOK], F32, kind="ExternalOutput")
        dbg_names.append('dbgx')

    with tile.TileContext(nc) as tc:
        _build_tc(nc, tc, tens, out_t,
                  dbgx_t if dbg_names else None)
    return dbg_names


def _build_tc(nc, tc, tens, out_t, dbgx_t):
    ctx_pools = {}

    def pool(name, bufs, space="SBUF"):
        if name not in ctx_pools:
            ctx_pools[name] = tc.alloc_tile_pool(name=name, bufs=bufs, space=space)
        return ctx_pools[name]

    state = pool("state", 1)
    ppool = {  # 3 + 4 + 1 = 8 PSUM banks
        'pbig': pool("psbig", 4, "PSUM"),
        'ps': pool("psattn", 3, "PSUM"),
        'prs': pool("psrs", 1, "PSUM"),
    }
    dpool = pool("dram", 2, "DRAM")

    # ---- persistent tiles
    x = state.tile([128, PT, TOK], F32, tag="x")
    consts_bf = state.tile([128, 808], BF16, tag="cbf")
    consts_f = state.tile([128, 136], F32, tag="cf")
    biases = state.tile([128, DEPTH, 54], F32, tag="biases")
    nc.sync.dma_start(out=consts_bf[:], in_=tens['consts_bf'].ap())
    nc.sync.dma_start(out=consts_f[:], in_=tens['consts_f'].ap())
    nc.sync.dma_start(out=biases[:], in_=tens['biases'].ap())

    ones_bf = consts_bf[:, 64:65]          # [128,1] bf16
    identf = consts_f[:, 0:128]            # [128,128] f32 I128
    onesf = consts_f[:, 128:129]           # [128,1] f32
    eps_ap = consts_f[0:1, 129:130]        # [1,1] f32 = EPS

    # ---- mask / scalar state (ping-pong pools)
    mstate = pool("mstate", 2)
    mb_k0 = mstate.tile([128, 1], F32, tag="mbk0")
    mb_k1 = mstate.tile([69, 1], F32, tag="mbk1")
    mrow_bias = mstate.tile([1, T], F32, tag="mrowb")
    mask_row = mstate.tile([1, NPATCH], F32, tag="maskrow")
    nf = mstate.tile([1, 1], F32, tag="nf")
    pmass = mstate.tile([1, 1], F32, tag="pmass")
    pvalid = mstate.tile([1, 1], F32, tag="pvalid")
    nc.vector.memset(mb_k0[:], 0.0)
    nc.vector.memset(mb_k1[:], 0.0)
    nc.vector.memset(mrow_bias[:], 0.0)
    nc.vector.memset(mask_row[:], 1.0)
    nc.vector.memset(nf[:], float(NPATCH))
    nc.vector.memset(pmass[:], 0.0)
    nc.vector.memset(pvalid[:], 0.0)

    # ---- embed: x[:, j, item*197 + 1 + n] = patches @ wpatch + pos/bias; CLS col
    with tc.tile_pool(name="embed", bufs=1) as ep:
        wpatch = ep.tile([128, PT, D], BF16, tag="wpatch")
        nc.sync.dma_start(out=wpatch[:], in_=tens['wpatch'].ap().rearrange("j p f -> p j f"))
        xt = ep.tile([128, PT, BL * NPATCH], BF16, tag="xt")
        nc.sync.dma_start(out=xt[:], in_=tens['xt'].ap())
        pos_cls = ep.tile([128, PT, T + 1], F32, tag="poscls")
        nc.sync.dma_start(out=pos_cls[:], in_=tens['pos_cls'].ap())
        for b in range(BL):
            for fo in range(PT):
                pe = ppool['pbig'].tile([128, NPATCH], F32, tag="pbig")
                for j in range(PT):
                    nc.tensor.matmul(pe[:], wpatch[:, j, fo * 128:(fo + 1) * 128],
                                     xt[:, j, b * NPATCH:(b + 1) * NPATCH],
                                     start=(j == 0), stop=(j == PT - 1))
                nc.vector.tensor_tensor(out=x[:, fo, b * T + 1:(b + 1) * T],
                                        in0=pe[:], in1=pos_cls[:, fo, 1:197], op=OP.add)
                nc.vector.tensor_copy(out=x[:, fo, b * T:b * T + 1], in_=pos_cls[:, fo, 0:1])

    # ---- transformer layers
    for l in range(DEPTH_BUILD):
        mb_k0, mb_k1, mrow_bias, mask_row, nf, pmass, pvalid = _layer(
            nc, tc, pool, tens, l, x, biases, ones_bf, consts_bf, identf, onesf, eps_ap,
            mb_k0, mb_k1, mrow_bias, mask_row, nf, pmass, pvalid)

    if dbgx_t is not None:
        nc.sync.dma_start(out=dbgx_t.ap(), in_=x[:])

    # ---- final LN on CLS tokens + head
    hp = pool("head", 1)
    xcls = hp.tile([128, PT, BL], F32, tag="xcls")
    src = bass.AP(tensor=x.tensor, offset=x[:].offset,
                  ap=[x[:].ap[0], [TOK, PT], [T, BL]])
    nc.vector.tensor_copy(out=xcls[:], in_=src)
    xcb = hp.tile([128, PT, BL], BF16, tag="xcb")
    x2b = hp.tile([128, PT, BL], BF16, tag="x2b")
    nc.vector.tensor_copy(out=xcb[:], in_=xcls[:])
    nc.vector.tensor_tensor(out=x2b[:], in0=xcls[:], in1=xcls[:], op=OP.mult)
    ps_s = ppool['ps'].tile([1, BL], F32, tag="ps")
    for j in range(PT):
        nc.tensor.matmul(ps_s[:], ones_bf[:], xcb[:, j, :], start=(j == 0), stop=(j == PT - 1))
    ps_q = ppool['ps'].tile([1, BL], F32, tag="ps")
    for j in range(PT):
        nc.tensor.matmul(ps_q[:], ones_bf[:], x2b[:, j, :], start=(j == 0), stop=(j == PT - 1))
    mu = hp.tile([1, BL], F32, tag="hmu")
    msq = hp.tile([1, BL], F32, tag="hmsq")
    nc.vector.tensor_scalar(out=mu[:], in0=ps_s[:], scalar1=1.0 / D, scalar2=None, op0=OP.mult)
    nc.vector.tensor_scalar(out=msq[:], in0=ps_q[:], scalar1=1.0 / D, scalar2=None, op0=OP.mult)
    var = hp.tile([1, BL], F32, tag="hvar")
    nc.vector.tensor_tensor(out=var[:], in0=mu[:], in1=mu[:], op=OP.mult)
    nc.vector.tensor_tensor(out=var[:], in0=msq[:], in1=var[:], op=OP.subtract)
    sdh = hp.tile([1, BL], F32, tag="hsd")
    nc.scalar.activation(out=sdh[:], in_=var[:], func=AF.Sqrt, bias=eps_ap, scale=1.0)
    rr = hp.tile([1, BL], F32, tag="hr")
    nc.vector.reciprocal(out=rr[:], in_=sdh[:])
    mu_b = hp.tile([128, BL], F32, tag="hmub")
    r_b = hp.tile([128, BL], F32, tag="hrb")
    nc.gpsimd.partition_broadcast(mu_b[:], mu[:], channels=128)
    nc.gpsimd.partition_broadcast(r_b[:], rr[:], channels=128)
    hcls = hp.tile([128, PT, BL], BF16, tag="hcls")
    for j in range(PT):
        tmp = hp.tile([128, BL], F32, tag="htmp")
        nc.vector.tensor_tensor(out=tmp[:], in0=xcls[:, j, :], in1=mu_b[:], op=OP.subtract)
        nc.vector.tensor_tensor(out=hcls[:, j, :], in0=tmp[:], in1=r_b[:], op=OP.mult)
    bhead = hp.tile([128, 8], F32, tag="bhead")
    nc.sync.dma_start(out=bhead[:], in_=tens['bhead'].ap())
    wh = None
    for fo in range(8):
        pw = min(128, NCLS - fo * 128)
        c0 = (fo // 2) * 256
        if fo % 2 == 0:
            wh = hp.tile([128, PT, 256], BF16, tag="whead")
            nc.sync.dma_start(
                out=wh[:, :, 0:min(256, NCLS - c0)],
                in_=tens['whead'].ap().rearrange("j p f -> p j f")[:, :, c0:min(c0 + 256, NCLS)])
        co = fo * 128 - c0
        po = ppool['ps'].tile([128, BL], F32, tag="ps")
        for j in range(PT):
            nc.tensor.matmul(po[0:pw, :], wh[:, j, co:co + pw],
                             hcls[:, j, :], start=(j == 0), stop=(j == PT - 1))
        osb = hp.tile([128, BL], F32, tag="osb")
        nc.vector.tensor_scalar(out=osb[0:pw, :], in0=po[0:pw, :],
                                scalar1=bhead[0:pw, fo:fo + 1], scalar2=None, op0=OP.add)
        nc.sync.dma_start(
            out=out_t.ap()[:, fo * 128:fo * 128 + pw].rearrange("b p -> p b"),
            in_=osb[0:pw, :])

    for pname in reversed(list(ctx_pools)):
        ctx_pools[pname].release()


def _ln(nc, tc, pool, x, tcix, ones_bf, eps_ap, h_out, ppool):
    """LayerNorm (no affine) of x[:, :, chunk] -> h_out bf16 [128, PT, TCW].

    Casts/squares run on DVE (keeps the scalar engine's activation table
    stable); rsqrt(var+eps) is a single scalar op."""
    sl = slice(tcix * TCW, (tcix + 1) * TCW)
    lc = pool("lncast", 3)
    lr = pool("lnrow", 3)
    lb = pool("lnb", 2)
    lt = pool("lntmp", 1)
    ps_s = ppool['pbig'].tile([1, TCW], F32, tag="pbig")
    ps_q = ppool['pbig'].tile([1, TCW], F32, tag="pbig")
    for j in range(PT):
        xb = lc.tile([128, TCW], BF16, tag="lncast")
        x2 = lc.tile([128, TCW], BF16, tag="lncast")
        nc.vector.tensor_copy(out=xb[:], in_=x[:, j, sl])
        nc.scalar.activation(out=x2[:], in_=x[:, j, sl], func=AF.Square, bias=0.0, scale=1.0)
        nc.tensor.matmul(ps_s[:], ones_bf[:], xb[:], start=(j == 0), stop=(j == PT - 1))
        nc.tensor.matmul(ps_q[:], ones_bf[:], x2[:], start=(j == 0), stop=(j == PT - 1))
    mu = lr.tile([1, TCW], F32, tag="lnrow")
    msq = lr.tile([1, TCW], F32, tag="lnrow")
    nc.vector.tensor_scalar(out=mu[:], in0=ps_s[:], scalar1=1.0 / D, scalar2=None, op0=OP.mult)
    nc.vector.tensor_scalar(out=msq[:], in0=ps_q[:], scalar1=1.0 / D, scalar2=None, op0=OP.mult)
    var = lr.tile([1, TCW], F32, tag="lnrow")
    nc.vector.tensor_tensor(out=var[:], in0=mu[:], in1=mu[:], op=OP.mult)
    nc.vector.tensor_tensor(out=var[:], in0=msq[:], in1=var[:], op=OP.subtract)
    sd = lr.tile([1, TCW], F32, tag="lnrow")
    nc.scalar.activation(out=sd[:], in_=var[:], func=AF.Sqrt, bias=eps_ap, scale=1.0)
    rr = lr.tile([1, TCW], F32, tag="lnrow")
    nc.vector.reciprocal(out=rr[:], in_=sd[:])
    mu_b = lb.tile([128, TCW], F32, tag="lnb")
    r_b = lb.tile([128, TCW], F32, tag="lnb")
    nc.gpsimd.partition_broadcast(mu_b[:], mu[:], channels=128)
    nc.gpsimd.partition_broadcast(r_b[:], rr[:], channels=128)
    for j in range(PT):
        tmp = lt.tile([128, TCW], F32, tag="lntmp")
        nc.vector.tensor_tensor(out=tmp[:], in0=x[:, j, sl], in1=mu_b[:], op=OP.subtract)
        nc.vector.tensor_tensor(out=h_out[:, j, :], in0=tmp[:], in1=r_b[:], op=OP.mult)


def _layer(nc, tc, pool, tens, l, x, biases, ones_bf, consts_bf, identf, onesf, eps_ap,
           mb_k0, mb_k1, mrow_bias, mask_row, nf, pmass, pvalid):
    ppool = {'pbig': pool("psbig", 4, "PSUM"), 'ps': pool("psattn", 3, "PSUM"),
             'prs': pool("psrs", 1, "PSUM")}
    wpool = pool("w", 1)
    dpool = pool("dram", 2, "DRAM")
    mp_ = pool("mask", 2)
    mtr = pool("mtrow", 3)
    ap1 = pool("attnbig", 1)
    php = pool("pth", 1)
    asm = pool("attnsm", 1)
    rbp = pool("rbp", 1)
    b_qkv = lambda fo: biases[:, l, fo:fo + 1]
    b_proj = lambda fo: biases[:, l, 18 + fo:18 + fo + 1]
    b_fc1 = lambda fo: biases[:, l, 24 + fo:24 + fo + 1]
    b_fc2 = lambda fo: biases[:, l, 48 + fo:48 + fo + 1]
    ident128 = consts_bf[:, 80:208]
    rsel = lambda hh: consts_bf[:, 208 + 12 * hh:208 + 12 * hh + 12]
    vsel = lambda j: consts_bf[:, 352 + 12 * j:352 + 12 * j + 12]

    # wqkv <-> wfc1 and wproj <-> wfc2 time-share SBUF (never alive together)
    wqkv_t = wpool.tile([128, PT, FF], BF16, tag="wbig1")
    wqkv = wqkv_t[:, :, 0:3 * D]
    nc.sync.dma_start(out=wqkv, in_=tens['wqkv'].ap()[l].rearrange("j p f -> p j f"))
    wproj_t = wpool.tile([128, FT, D], BF16, tag="wbig2")
    wproj = wproj_t[:, 0:PT, :]
    nc.sync.dma_start(out=wproj, in_=tens['wproj'].ap()[l].rearrange("j p f -> p j f"))

    mrow_b12 = asm.tile([12, T], F32, tag="mrowb12")
    nc.gpsimd.partition_broadcast(mrow_b12[:], mrow_bias[:], channels=12)
    imp12 = asm.tile([12, T], F32, tag="imp12")
    first_imp = [True]

    for tcix in range(NTC):
        # ---- LN1 + qkv
        h = pool("h", 1).tile([128, PT, TCW], BF16, tag="h")
        _ln(nc, tc, pool, x, tcix, ones_bf, eps_ap, h, ppool)
        qkv = pool("qkvsb", 1).tile([128, QT, TCW], BF16, tag="qkv")
        for fo in range(QT):
            pq = ppool['pbig'].tile([128, TCW], F32, tag="pbig")
            for j in range(PT):
                nc.tensor.matmul(pq[:], wqkv[:, j, fo * 128:(fo + 1) * 128], h[:, j, :],
                                 start=(j == 0), stop=(j == PT - 1))
            nc.vector.tensor_scalar(out=qkv[:, fo, :], in0=pq[:], scalar1=b_qkv(fo),
                                    scalar2=None, op0=OP.add)
        av_t = pool("gav", 1).tile([128, FT, TCW], BF16, tag="gav")
        av = av_t[:, 0:PT, :]

        # ---- vT per ptile per item (PE transposes)
        vt0 = ap1.tile([128, 2, D], BF16, tag="vt0")
        vt1 = ap1.tile([69, 2, D], BF16, tag="vt1")
        for b in range(2):
            for j in range(PT):
                tp0 = ppool['ps'].tile([128, 128], BF16, tag="ps")
                nc.tensor.transpose(tp0[:], qkv[:, 12 + j, b * T:b * T + 128], ident128)
                nc.vector.tensor_copy(out=vt0[:, b, j * 128:(j + 1) * 128], in_=tp0[:])
                tp1 = ppool['ps'].tile([69, 128], BF16, tag="ps")
                nc.tensor.transpose(tp1[:], qkv[:, 12 + j, b * T + 128:(b + 1) * T], ident128)
                nc.vector.tensor_copy(out=vt1[:, b, j * 128:(j + 1) * 128], in_=tp1[:])

        # ---- vnorm rows, all heads: [12, TCW]
        pvn = ppool['ps'].tile([12, TCW], F32, tag="ps")
        for j in range(PT):
            v2 = asm.tile([128, TCW], BF16, tag="v2")
            nc.vector.tensor_tensor(out=v2[:], in0=qkv[:, 12 + j, :], in1=qkv[:, 12 + j, :],
                                    op=OP.mult)
            nc.tensor.matmul(pvn[:], vsel(j), v2[:], start=(j == 0), stop=(j == PT - 1))
        vnr = asm.tile([12, TCW], F32, tag="vnr")
        nc.scalar.activation(out=vnr[:], in_=pvn[:], func=AF.Sqrt, bias=0.0, scale=1.0)

        # ---- CLS attention rows, all heads: q_cls one-hot stationary + 12 MMs
        qcb = asm.tile([128, PT, 24], BF16, tag="qcb")
        nc.vector.memset(qcb[:], 0.0)
        for b in range(2):
            for j in range(PT):
                c = b * 12 + 2 * j
                nc.vector.tensor_copy(out=qcb[0:64, j, c:c + 1],
                                      in_=qkv[0:64, j, b * T:b * T + 1])
                nc.vector.tensor_copy(out=qcb[64:128, j, c + 1:c + 2],
                                      in_=qkv[64:128, j, b * T:b * T + 1])
        pcls = ppool['ps'].tile([12, TCW], F32, tag="ps")
        for b in range(2):
            for j in range(PT):
                nc.tensor.matmul(pcls[:, b * T:(b + 1) * T], qcb[:, j, b * 12:b * 12 + 12],
                                 qkv[:, 6 + j, b * T:(b + 1) * T],
                                 start=(j == 0), stop=(j == PT - 1))
        for b in range(2):
            crow = asm.tile([12, T], F32, tag="crow")
            nc.vector.tensor_tensor(out=crow[:], in0=pcls[:, b * T:(b + 1) * T],
                                    in1=mrow_b12[:], op=OP.add)
            erow = asm.tile([12, T], F32, tag="erow")
            esum = asm.tile([12, 1], F32, tag="esum")
            nc.scalar.activation(out=erow[:], in_=crow[:], func=AF.Exp, bias=0.0, scale=1.0,
                                 accum_out=esum[:])
            ercp = asm.tile([12, 1], F32, tag="ercp")
            nc.vector.reciprocal(out=ercp[:], in_=esum[:])
            contrib = asm.tile([12, T], F32, tag="contrib")
            nc.vector.scalar_tensor_tensor(out=contrib[:], in0=erow[:], scalar=ercp[:],
                                           in1=vnr[:, b * T:(b + 1) * T],
                                           op0=OP.mult, op1=OP.mult)
            if first_imp[0]:
                nc.vector.tensor_copy(out=imp12[:], in_=contrib[:])
                first_imp[0] = False
            else:
                nc.vector.tensor_tensor(out=imp12[:], in0=imp12[:], in1=contrib[:], op=OP.add)

        # ---- scores / exp / row-sums / av, in two 6-head half-groups: av of
        # heads 0-5 overlaps scoring of heads 6-11, and pt buffers are halved
        # rsel6(hh): [128, 6] one-hot column hh%6 (sub-slice of the rsel block)
        rsel6 = lambda hh: consts_bf[:, 208 + 12 * hh + 6 * (hh // 6):
                                     208 + 12 * hh + 6 * (hh // 6) + 6]
        for hg in range(2):
            pt0 = php.tile([128, 6, TCW], BF16, tag="pt0")
            pt1 = php.tile([69, 6, TCW], BF16, tag="pt1")
            prs_h = ppool['prs'].tile([6, TCW], F32, tag="prs")

            def _scores(hh):
                hl = hh % 6
                bsl = slice((hh % 2) * 64, (hh % 2) * 64 + 64)
                jq = hh // 2
                psc0 = ppool['ps'].tile([128, TCW], F32, tag="ps")
                psc1 = ppool['ps'].tile([69, TCW], F32, tag="ps")
                for b in range(2):
                    q_ap = qkv[bsl, jq, b * T:(b + 1) * T]
                    nc.tensor.matmul(psc0[:, b * T:(b + 1) * T],
                                     qkv[bsl, 6 + jq, b * T:b * T + 128], q_ap,
                                     start=True, stop=True)
                    nc.tensor.matmul(psc1[:, b * T:(b + 1) * T],
                                     qkv[bsl, 6 + jq, b * T + 128:(b + 1) * T], q_ap,
                                     start=True, stop=True)
                nc.scalar.activation(out=pt0[:, hl, :], in_=psc0[:], func=AF.Exp,
                                     bias=mb_k0[:], scale=1.0)
                nc.scalar.activation(out=pt1[:, hl, :], in_=psc1[:], func=AF.Exp,
                                     bias=mb_k1[0:69, :], scale=1.0)

            def _rowsum(hh):
                hl = hh % 6
                nc.tensor.matmul(prs_h[:], rsel6(hh), pt0[:, hl, :],
                                 start=(hl == 0), stop=False)
                nc.tensor.matmul(prs_h[:], rsel6(hh)[0:69, :], pt1[:, hl, :],
                                 start=False, stop=(hl == 5))

            for hh in range(hg * 6, hg * 6 + 6):
                _scores(hh)
                if hh % 6 >= 1:
                    _rowsum(hh - 1)
            _rowsum(hg * 6 + 5)

            # softmax normalize + av for this half's 3 ptiles; the per-head
            # 1/rowsum rows are replicated across their 64 feature partitions
            # by a tiny PE matmul against block-select constants
            rrow6 = php.tile([6, TCW], BF16, tag="rrow6")
            with nc.allow_low_precision(reason="softmax 1/rowsum in bf16 for PE row-bcast"):
                nc.vector.reciprocal(out=rrow6[:], in_=prs_h[:])
            for p in range(hg * 3, hg * 3 + 3):
                pl = p - hg * 3
                rbps = ppool['ps'].tile([128, TCW], F32, tag="ps")
                nc.tensor.matmul(rbps[:], consts_bf[0:6, 424 + pl * 128:424 + (pl + 1) * 128],
                                 rrow6[:], start=True, stop=True)
                rb = rbp.tile([128, TCW], F32, tag="rb")
                nc.vector.tensor_copy(out=rb[:], in_=rbps[:])
                pav = ppool['ps'].tile([128, TCW], F32, tag="ps")
                for b in range(2):
                    tsl = slice(b * T, (b + 1) * T)
                    for hi in range(2):
                        hh = 2 * p + hi
                        hl = hh % 6
                        osl = slice(hi * 64, hi * 64 + 64)
                        nc.tensor.matmul(pav[osl, tsl], vt0[:, b, hh * 64:(hh + 1) * 64],
                                         pt0[:, hl, tsl], start=True, stop=False)
                        nc.tensor.matmul(pav[osl, tsl], vt1[:, b, hh * 64:(hh + 1) * 64],
                                         pt1[:, hl, tsl], start=False, stop=True)
                nc.vector.tensor_tensor(out=av[:, p, :], in0=pav[:], in1=rb[:], op=OP.mult)

        # ---- proj + residual
        for fo in range(PT):
            pp = ppool['pbig'].tile([128, TCW], F32, tag="pbig")
            for j in range(PT):
                nc.tensor.matmul(pp[:], wproj[:, j, fo * 128:(fo + 1) * 128], av[:, j, :],
                                 start=(j == 0), stop=(j == PT - 1))
            sl = slice(tcix * TCW, (tcix + 1) * TCW)
            nc.vector.scalar_tensor_tensor(out=x[:, fo, sl], in0=pp[:], scalar=b_proj(fo),
                                           in1=x[:, fo, sl], op0=OP.add, op1=OP.add)

    # ---- importance: reduce heads, scale, start AllReduce (result consumed
    # after the MLP so the collective latency hides under GEMMs)
    pimp = ppool['ps'].tile([1, T], F32, tag="ps")
    nc.tensor.matmul(pimp[:], onesf[0:12, :], imp12[:], start=True, stop=True)
    impw = mtr.tile([1, NPATCH], F32, tag="mtrow")
    nc.vector.tensor_scalar(out=impw[:], in0=pimp[0:1, 1:T], scalar1=1.0 / (B * H),
                            scalar2=None, op0=OP.mult)
    in_b = dpool.tile([1, NPATCH], F32, tag="ccin")
    out_b = dpool.tile([1, NPATCH], F32, tag="ccout")
    nc.gpsimd.dma_start(out=in_b[:], in_=impw[:])
    nc.gpsimd.collective_compute(
        "AllReduce", OP.add, replica_groups=[list(range(NCORE))],
        ins=[in_b[:].opt()], outs=[out_b[:].opt()])

    # ---- MLP
    wfc1 = wpool.tile([128, PT, FF], BF16, tag="wbig1")
    nc.sync.dma_start(out=wfc1[:], in_=tens['wfc1'].ap()[l].rearrange("j p f -> p j f"))
    wfc2 = wpool.tile([128, FT, D], BF16, tag="wbig2")
    nc.sync.dma_start(out=wfc2[:], in_=tens['wfc2'].ap()[l].rearrange("j p f -> p j f"))
    def _mask_chain():
        imp_g = mp_.tile([1, NPATCH], F32, tag="impg")
        nc.gpsimd.dma_start(out=imp_g[:], in_=out_b[:])

        mass = mp_.tile([1, 1], F32, tag="mass")
        nc.vector.tensor_reduce(out=mass[:], in_=imp_g[:], axis=AX, op=OP.add)
        me = mp_.tile([1, 1], F32, tag="me")
        nc.vector.tensor_scalar(out=me[:], in0=mass[:], scalar1=EPS, scalar2=None, op0=OP.add)
        mrec = mp_.tile([1, 1], F32, tag="mrec")
        nc.vector.reciprocal(out=mrec[:], in_=me[:])
        p_r = mtr.tile([1, NPATCH], F32, tag="mtrow")
        nc.vector.tensor_scalar(out=p_r[:], in0=imp_g[:], scalar1=mrec[:], scalar2=None, op0=OP.mult)
        lnp = mtr.tile([1, NPATCH], F32, tag="mtrow")
        nc.scalar.activation(out=lnp[:], in_=p_r[:], func=AF.Ln, bias=eps_ap, scale=1.0)
        pl = mtr.tile([1, NPATCH], F32, tag="mtrow")
        nc.vector.tensor_tensor(out=pl[:], in0=p_r[:], in1=lnp[:], op=OP.mult)
        entn = mp_.tile([1, 1], F32, tag="entn")     # = -entropy
        nc.vector.tensor_reduce(out=entn[:], in_=pl[:], axis=AX, op=OP.add)
        lnN = mp_.tile([1, 1], F32, tag="lnN")
        nc.scalar.activation(out=lnN[:], in_=nf[:], func=AF.Ln, bias=0.0, scale=1.0)
        lrec = mp_.tile([1, 1], F32, tag="lrec")
        nc.vector.reciprocal(out=lrec[:], in_=lnN[:])
        nrho = mp_.tile([1, 1], F32, tag="nrho")     # = -rho
        nc.vector.tensor_tensor(out=nrho[:], in0=entn[:], in1=lrec[:], op=OP.mult)
        pme = mp_.tile([1, 1], F32, tag="pme")
        nc.vector.tensor_scalar(out=pme[:], in0=pmass[:], scalar1=EPS, scalar2=None, op0=OP.add)
        pmr = mp_.tile([1, 1], F32, tag="pmr")
        nc.vector.reciprocal(out=pmr[:], in_=pme[:])
        ratio = mp_.tile([1, 1], F32, tag="ratio")
        nc.vector.tensor_tensor(out=ratio[:], in0=mass[:], in1=pmr[:], op=OP.mult)
        nrr = mp_.tile([1, 1], F32, tag="nrr")
        nc.vector.tensor_tensor(out=nrr[:], in0=nrho[:], in1=ratio[:], op=OP.mult)
        kr = mp_.tile([1, 1], F32, tag="kr")
        nc.vector.tensor_scalar(out=kr[:], in0=nrr[:], scalar1=GAMMA, scalar2=1.0,
                                op0=OP.mult, op1=OP.add)
        nc.vector.tensor_scalar(out=kr[:], in0=kr[:], scalar1=1.0, scalar2=0.0,
                                op0=OP.min, op1=OP.max)
        pvc = mp_.tile([1, 1], F32, tag="pvc")
        nc.vector.tensor_scalar(out=pvc[:], in0=pvalid[:], scalar1=-1.0, scalar2=1.0,
                                op0=OP.mult, op1=OP.add)
        krm = mp_.tile([1, 1], F32, tag="krm")
        nc.vector.scalar_tensor_tensor(out=krm[:], in0=kr[:], scalar=pvalid[:], in1=pvc[:],
                                       op0=OP.mult, op1=OP.add)
        y = mp_.tile([1, 1], F32, tag="y")
        nc.vector.tensor_tensor(out=y[:], in0=nf[:], in1=krm[:], op=OP.mult)
        m_th = mp_.tile([1, 1], F32, tag="m_th")
        nc.vector.tensor_tensor(out=m_th[:], in0=y[:], in1=nf[:], op=OP.min)
        nc.vector.tensor_scalar(out=m_th[:], in0=m_th[:], scalar1=-1.0, scalar2=None, op0=OP.add)

        # ranks: imp_r = imp_g + (mask-1)*1e30 ; rank[p] = #{f: imp_r[f] > imp_r[p]}
        pen = mtr.tile([1, NPATCH], F32, tag="mtrow")
        nc.vector.tensor_scalar(out=pen[:], in0=mask_row[:], scalar1=1e30, scalar2=-1e30,
                                op0=OP.mult, op1=OP.add)
        imp_r = mp_.tile([1, NPATCH], F32, tag="impr")
        nc.vector.tensor_tensor(out=imp_r[:], in0=imp_g[:], in1=pen[:], op=OP.add)
        # columns via K=1 f32 matmuls
        c0p = ppool['ps'].tile([128, 1], F32, tag="ps")
        nc.tensor.matmul(c0p[:], imp_r[0:1, 0:128], onesf[0:1, :], start=True, stop=True)
        c1p = ppool['ps'].tile([68, 1], F32, tag="ps")
        nc.tensor.matmul(c1p[:], imp_r[0:1, 128:NPATCH], onesf[0:1, :], start=True, stop=True)
        c0 = mp_.tile([128, 1], F32, tag="c0")
        c1 = mp_.tile([68, 1], F32, tag="c1")
        nc.vector.tensor_copy(out=c0[:], in_=c0p[:])
        nc.vector.tensor_copy(out=c1[:], in_=c1p[:])
        ib0 = pool("mbig", 1).tile([128, NPATCH], F32, tag="ib0")
        nc.gpsimd.partition_broadcast(ib0[:], imp_r[:], channels=128)
        junk = pool("mbig", 1).tile([128, NPATCH], F32, tag="junkr")
        rk0 = mp_.tile([128, 1], F32, tag="rk0")
        nc.vector.tensor_scalar(out=junk[:], in0=ib0[:], scalar1=c0[:], scalar2=0.0,
                                op0=OP.is_gt, op1=OP.add, accum_out=rk0[:])
        rk1 = mp_.tile([68, 1], F32, tag="rk1")
        nc.vector.tensor_scalar(out=junk[0:68, :], in0=ib0[0:68, :], scalar1=c1[:], scalar2=0.0,
                                op0=OP.is_gt, op1=OP.add, accum_out=rk1[:])
        mthb = mp_.tile([128, 1], F32, tag="mthb")
        nc.gpsimd.partition_broadcast(mthb[:], m_th[:], channels=128)
        ca0 = mp_.tile([128, 1], F32, tag="ca0")
        nc.vector.tensor_scalar(out=ca0[:], in0=rk0[:], scalar1=15.5, scalar2=None, op0=OP.is_lt)
        cb0 = mp_.tile([128, 1], F32, tag="cb0")
        nc.vector.tensor_scalar(out=cb0[:], in0=rk0[:], scalar1=mthb[:], scalar2=None, op0=OP.is_le)
        nmask0 = mp_.tile([128, 1], F32, tag="nmask0")
        nc.vector.tensor_tensor(out=nmask0[:], in0=ca0[:], in1=cb0[:], op=OP.max)
        ca1 = mp_.tile([68, 1], F32, tag="ca1")
        nc.vector.tensor_scalar(out=ca1[:], in0=rk1[:], scalar1=15.5, scalar2=None, op0=OP.is_lt)
        cb1 = mp_.tile([68, 1], F32, tag="cb1")
        nc.vector.tensor_scalar(out=cb1[:], in0=rk1[:], scalar1=mthb[0:68, :], scalar2=None,
                                op0=OP.is_le)
        nmask1 = mp_.tile([68, 1], F32, tag="nmask1")
        nc.vector.tensor_tensor(out=nmask1[:], in0=ca1[:], in1=cb1[:], op=OP.max)

        # mask columns -> row via PE transpose (f32)
        mr0p = ppool['ps'].tile([1, 128], F32, tag="ps")
        nc.tensor.transpose(mr0p[:], nmask0[:], identf)
        mr1p = ppool['ps'].tile([1, 68], F32, tag="ps")
        nc.tensor.transpose(mr1p[:], nmask1[:], identf[0:68, 0:68])
        mask_row_n = mp_.tile([1, NPATCH], F32, tag="maskrow")
        nc.vector.tensor_copy(out=mask_row_n[0:1, 0:128], in_=mr0p[:])
        nc.vector.tensor_copy(out=mask_row_n[0:1, 128:NPATCH], in_=mr1p[:])

        # key-space additive bias state for next layer (CLS row = 0 bias)
        mkb = dpool.tile([1, T], F32, tag="mkb")
        nc.gpsimd.dma_start(out=mkb[0:1, 0:1], in_=onesf[0:1, :])
        nc.gpsimd.dma_start(out=mkb[0:1, 1:T], in_=mask_row_n[:])
        mb_k0_n = mp_.tile([128, 1], F32, tag="mbk0")
        mb_k1_n = mp_.tile([69, 1], F32, tag="mbk1")
        nc.gpsimd.dma_start(out=mb_k0_n[:], in_=mkb[0, 0:128].unsqueeze(1))
        nc.gpsimd.dma_start(out=mb_k1_n[:], in_=mkb[0, 128:T].unsqueeze(1))
        nc.vector.tensor_scalar(out=mb_k0_n[:], in0=mb_k0_n[:], scalar1=MB, scalar2=-MB,
                                op0=OP.mult, op1=OP.add)
        nc.vector.tensor_scalar(out=mb_k1_n[:], in0=mb_k1_n[:], scalar1=MB, scalar2=-MB,
                                op0=OP.mult, op1=OP.add)
        mrow_bias_n = mp_.tile([1, T], F32, tag="mrowb")
        nc.vector.memset(mrow_bias_n[0:1, 0:1], 0.0)
        nc.vector.tensor_scalar(out=mrow_bias_n[0:1, 1:T], in0=mask_row_n[:], scalar1=MB,
                                scalar2=-MB, op0=OP.mult, op1=OP.add)

        nf_n = mp_.tile([1, 1], F32, tag="nf")
        nc.vector.tensor_reduce(out=nf_n[:], in_=mask_row_n[:], axis=AX, op=OP.add)
        pvalid_n = mp_.tile([1, 1], F32, tag="pvalid")
        nc.vector.tensor_scalar(out=pvalid_n[:], in0=nf[:], scalar1=16.5, scalar2=None, op0=OP.is_gt)
        pmass_n = mp_.tile([1, 1], F32, tag="pmass")
        nc.vector.tensor_copy(out=pmass_n[:], in_=mass[:])
        return mb_k0_n, mb_k1_n, mrow_bias_n, mask_row_n, nf_n, pmass_n, pvalid_n

    mask_res = [None]
    for tcix in range(NTC):
        h2 = pool("h", 1).tile([128, PT, TCW], BF16, tag="h")
        _ln(nc, tc, pool, x, tcix, ones_bf, eps_ap, h2, ppool)
        g = pool("gav", 1).tile([128, FT, TCW], BF16, tag="gav")
        for fo in range(FT):
            pf = ppool['pbig'].tile([128, TCW], F32, tag="pbig")
            for j in range(PT):
                nc.tensor.matmul(pf[:], wfc1[:, j, fo * 128:(fo + 1) * 128], h2[:, j, :],
                                 start=(j == 0), stop=(j == PT - 1))
            nc.scalar.activation(out=g[:, fo, :], in_=pf[:], func=AF.Gelu,
                                 bias=b_fc1(fo), scale=1.0)
        sl = slice(tcix * TCW, (tcix + 1) * TCW)
        for fo in range(PT):
            pf2 = ppool['pbig'].tile([128, TCW], F32, tag="pbig")
            for k in range(FT):
                nc.tensor.matmul(pf2[:], wfc2[:, k, fo * 128:(fo + 1) * 128], g[:, k, :],
                                 start=(k == 0), stop=(k == FT - 1))
            nc.vector.scalar_tensor_tensor(out=x[:, fo, sl], in0=pf2[:], scalar=b_fc2(fo),
                                           in1=x[:, fo, sl], op0=OP.add, op1=OP.add)
        if tcix == 2:
            # mask chain drains on the vector queue under chunk 3's GEMMs
            mask_res[0] = _mask_chain()
    return mask_res[0]


# ---------------------------------------------------------------- entry point
_CACHE = {}


def _get_nc():
    key = (DEPTH_BUILD, bool(os.environ.get('KERNEL_DEBUG_X')))
    if key not in _CACHE:
        nc = bacc.Bacc("TRN2", target_bir_lowering=False, debug=False, num_devices=NCORE)
        dbg = build(nc)
        nc.compile()
        _CACHE[key] = (nc, dbg)
    return _CACHE[key]


def kernel(**inputs):
    nc, dbg = _get_nc()
    shared = prep_weights(inputs)
    in_maps = []
    for c in range(NCORE):
        m = dict(shared)
        m['xt'] = prep_x_shard(inputs['x'], c)
        in_maps.append(m)
    res = run_bass_kernel_spmd(nc, in_maps, core_ids=list(range(NCORE)),
                               trace=bool(os.environ.get('KERNEL_TRACE')))
    kernel.last_results = res
    out = np.concatenate([res.results[c]['out'] for c in range(NCORE)], axis=0)
    return out.astype(np.float32)



# revision 17
# speedup vs baseline: 1.2023x; 1.2023x over previous
"""AdaptiveJacobianPrunedViT (ViT-B/16, B=64) on 8 TRN2 NeuronCores.

Data-parallel: batch sharded 8 ways (8 items/core), weights replicated.
Per-layer token-importance mean over (B,H) is AllReduce-summed across cores so
all replicas prune identical token sets.

Device layout: feature-major activations x[d, t] (d on partitions, 6x128;
t = item*197 + n, 1576 tokens/core). All matmuls run with weights as the
stationary operand in natural [d_in, d_out] layout, activations moving.
bf16 matmul inputs, fp32 PSUM/residual/softmax-stats/importance.
LayerNorm affine (g,b) is folded into the following matmul's weights on host;
q is pre-scaled by HD**-0.5.

Attention is head-batched: softmax row-sums, CLS rows and vnorm rows for all
12 heads of an item live in [12, T] tiles (one exp/recip/sqrt per chunk
instead of one per head); v is transposed per-ptile; the mask/AllReduce chain
runs after the MLP so its latency hides under the next layer's GEMMs.
"""
import os
import sys

sys.path.insert(0, '/opt/trn_rl_repo')

import numpy as np
import ml_dtypes

import concourse.bass as bass
import concourse.tile as tile
from concourse import bacc, mybir
from concourse.bass_utils import run_bass_kernel_spmd

F32 = mybir.dt.float32
BF16 = mybir.dt.bfloat16
AX = mybir.AxisListType.X
OP = mybir.AluOpType
AF = mybir.ActivationFunctionType

# model dims
B = 64; IMG = 224; P = 16; G = 14; NPATCH = 196; T = 197
D = 768; H = 12; HD = 64; DEPTH = 12; FF = 3072; NCLS = 1000
GAMMA = 0.01; MIN_TOK = 16; EPS = 1e-6; SCALE = HD ** -0.5

NCORE = 8
BL = B // NCORE            # items per core = 8
TOK = BL * T               # tokens per core = 1576
PT = D // 128              # 6 feature ptiles
QT = 3 * D // 128          # 18 qkv feature ptiles
FT = FF // 128             # 24 ff ptiles
NTC = 4                    # token chunks
TCW = TOK // NTC           # 394 tokens per chunk (2 items)
MB = 80.0                  # additive mask bias magnitude (exp(-78) == 0-ish, in LUT range)

bf16 = ml_dtypes.bfloat16

DEPTH_BUILD = int(os.environ.get('KERNEL_DEPTH', str(DEPTH)))


# ---------------------------------------------------------------- host prep
def prep_weights(ii):
    """Fold LN affines + scale into weights; build device-layout arrays."""
    f32 = lambda a: np.ascontiguousarray(np.asarray(a, np.float32))
    out = {}

    ln1_g = f32(ii['ln1_g']); ln1_b = f32(ii['ln1_b'])
    ln2_g = f32(ii['ln2_g']); ln2_b = f32(ii['ln2_b'])
    qkv_w = f32(ii['qkv_w']); qkv_b = f32(ii['qkv_b'])
    proj_w = f32(ii['proj_w']); proj_b = f32(ii['proj_b'])
    fc1_w = f32(ii['fc1_w']); fc1_b = f32(ii['fc1_b'])
    fc2_w = f32(ii['fc2_w']); fc2_b = f32(ii['fc2_b'])

    wqkv = np.empty((DEPTH, PT, 128, 3 * D), bf16)
    bqkv = np.empty((128, DEPTH, QT), np.float32)
    wproj = np.empty((DEPTH, PT, 128, D), bf16)
    bproj = np.empty((128, DEPTH, PT), np.float32)
    wfc1 = np.empty((DEPTH, PT, 128, FF), bf16)
    bfc1 = np.empty((128, DEPTH, FT), np.float32)
    wfc2 = np.empty((DEPTH, FT, 128, D), bf16)
    bfc2 = np.empty((128, DEPTH, PT), np.float32)
    for l in range(DEPTH):
        w = qkv_w[l] * ln1_g[l][:, None]
        b = ln1_b[l] @ qkv_w[l] + qkv_b[l]
        w[:, :D] *= SCALE
        b[:D] *= SCALE
        wqkv[l] = w.reshape(PT, 128, 3 * D).astype(bf16)
        bqkv[:, l, :] = b.reshape(QT, 128).T
        wproj[l] = proj_w[l].reshape(PT, 128, D).astype(bf16)
        bproj[:, l, :] = proj_b[l].reshape(PT, 128).T
        w1 = fc1_w[l] * ln2_g[l][:, None]
        b1 = ln2_b[l] @ fc1_w[l] + fc1_b[l]
        wfc1[l] = w1.reshape(PT, 128, FF).astype(bf16)
        bfc1[:, l, :] = b1.reshape(FT, 128).T
        wfc2[l] = fc2_w[l].reshape(FT, 128, D).astype(bf16)
        bfc2[:, l, :] = fc2_b[l].reshape(PT, 128).T
    out['wqkv'] = wqkv; out['wproj'] = wproj; out['wfc1'] = wfc1; out['wfc2'] = wfc2
    out['biases'] = np.ascontiguousarray(
        np.concatenate([bqkv, bproj, bfc1, bfc2], axis=2))  # [128, 12, 54]

    # patch embed: [3*16*16, 768] ; pos_cls [128, 6, 198]
    patch_w = f32(ii['patch_w'])
    out['wpatch'] = np.ascontiguousarray(patch_w.reshape(PT, 128, D).astype(bf16))
    pos = f32(ii['pos_embed'])[0]            # [197, 768]
    cls0 = f32(ii['cls_token'])[0, 0] + pos[0]  # [768]
    patch_b = f32(ii['patch_b'])
    pc = np.empty((128, PT, T + 1), np.float32)
    posT = pos.T.reshape(PT, 128, T)         # [6,128,197]
    for j in range(PT):
        pc[:, j, 0] = cls0[j * 128:(j + 1) * 128]
        pc[:, j, 1:197] = posT[j, :, 1:] + patch_b[j * 128:(j + 1) * 128, None]
        pc[:, j, 197] = 0.0
    out['pos_cls'] = pc

    # final norm folded into head
    norm_g = f32(ii['norm_g']); norm_b = f32(ii['norm_b'])
    head_w = f32(ii['head_w']); head_b = f32(ii['head_b'])
    hw = head_w * norm_g[:, None]
    hb = norm_b @ head_w + head_b
    out['whead'] = np.ascontiguousarray(hw.reshape(PT, 128, NCLS).astype(bf16))
    bh = np.zeros((128, 8), np.float32)
    bh.reshape(-1, order='F')[:NCLS] = hb    # col fo holds hb[fo*128 : fo*128+128]
    out['bhead'] = bh

    # constants (bf16): stacked I64 | ones | I128 | rsel (12x one-hot-col
    # blocks for head row-sums) | vsel (6x head-pair select blocks) |
    # ones128 (all-ones 128x128 for broadcast LN stats)
    cbf = np.zeros((128, 936), np.float32)
    cbf[:, 0:64] = np.tile(np.eye(64, dtype=np.float32), (2, 1))
    cbf[:, 64] = 1.0
    cbf[:, 80:208] = np.eye(128, dtype=np.float32)
    for h in range(H):
        cbf[:, 208 + 13 * h] = 1.0               # rsel[:, h, h] = 1
    for j in range(PT):
        cbf[0:64, 352 + j * 12 + 2 * j] = 1.0    # vsel: head 2j <- partitions 0:64
        cbf[64:128, 352 + j * 12 + 2 * j + 1] = 1.0
    for pl in range(3):                           # bsel6: row-bcast select blocks
        cbf[2 * pl, 424 + pl * 128:424 + pl * 128 + 64] = 1.0
        cbf[2 * pl + 1, 424 + pl * 128 + 64:424 + (pl + 1) * 128] = 1.0
    cbf[:, 808:936] = 1.0                         # ones128
    out['consts_bf'] = cbf.astype(bf16)
    cf = np.zeros((128, 136), np.float32)
    cf[:, 0:128] = np.eye(128, dtype=np.float32)  # identf for col->row transposes
    cf[:, 128] = 1.0                              # ones column (f32)
    cf[:, 129] = EPS                              # eps column (all partitions)
    out['consts_f'] = cf
    return out


def prep_x_shard(x, core):
    """x [B,3,224,224] -> per-core patches, feature-major bf16 [128, 6, 8*196]."""
    xs = np.asarray(x, np.float32)[core * BL:(core + 1) * BL]
    p = xs.reshape(BL, 3, G, P, G, P).transpose(0, 2, 4, 1, 3, 5).reshape(BL, NPATCH, 3 * P * P)
    pT = p.reshape(BL * NPATCH, 3 * P * P).T        # [768, 1568]
    return np.ascontiguousarray(pT.reshape(PT, 128, BL * NPATCH).transpose(1, 0, 2).astype(bf16))


# ---------------------------------------------------------------- builder
def build(nc):
    dt_in = [
        ('xt', [128, PT, BL * NPATCH], BF16),
        ('wqkv', [DEPTH, PT, 128, 3 * D], BF16),
        ('wproj', [DEPTH, PT, 128, D], BF16),
        ('wfc1', [DEPTH, PT, 128, FF], BF16),
        ('wfc2', [DEPTH, FT, 128, D], BF16),
        ('biases', [128, DEPTH, 54], F32),
        ('wpatch', [PT, 128, D], BF16),
        ('pos_cls', [128, PT, T + 1], F32),
        ('whead', [PT, 128, NCLS], BF16),
        ('bhead', [128, 8], F32),
        ('consts_bf', [128, 936], BF16),
        ('consts_f', [128, 136], F32),
    ]
    tens = {n: nc.dram_tensor(n, s, d, kind="ExternalInput") for n, s, d in dt_in}
    out_t = nc.dram_tensor("out", [BL, NCLS], F32, kind="ExternalOutput")

    dbg_names = []
    if os.environ.get('KERNEL_DEBUG_X'):
        dbgx_t = nc.dram_tensor("dbgx", [128, PT, TOK], F32, kind="ExternalOutput")
        dbg_names.append('dbgx')

    with tile.TileContext(nc) as tc:
        _build_tc(nc, tc, tens, out_t,
                  dbgx_t if dbg_names else None)
    return dbg_names


def _build_tc(nc, tc, tens, out_t, dbgx_t):
    ctx_pools = {}

    def pool(name, bufs, space="SBUF"):
        if name not in ctx_pools:
            ctx_pools[name] = tc.alloc_tile_pool(name=name, bufs=bufs, space=space)
        return ctx_pools[name]

    state = pool("state", 1)
    ppool = {  # 3 + 4 + 1 = 8 PSUM banks
        'pbig': pool("psbig", 4, "PSUM"),
        'ps': pool("psattn", 3, "PSUM"),
        'prs': pool("psrs", 1, "PSUM"),
    }
    dpool = pool("dram", 2, "DRAM")

    # ---- persistent tiles
    x = state.tile([128, PT, TOK], F32, tag="x")
    consts_bf = state.tile([128, 936], BF16, tag="cbf")
    consts_f = state.tile([128, 136], F32, tag="cf")
    biases = state.tile([128, DEPTH, 54], F32, tag="biases")
    nc.sync.dma_start(out=consts_bf[:], in_=tens['consts_bf'].ap())
    nc.sync.dma_start(out=consts_f[:], in_=tens['consts_f'].ap())
    nc.sync.dma_start(out=biases[:], in_=tens['biases'].ap())

    ones_bf = consts_bf[:, 64:65]          # [128,1] bf16
    ones128 = consts_bf[:, 808:936]        # [128,128] bf16 all-ones
    identf = consts_f[:, 0:128]            # [128,128] f32 I128
    onesf = consts_f[:, 128:129]           # [128,1] f32
    eps_col = consts_f[:, 129:130]         # [128,1] f32 = EPS
    eps_ap = consts_f[0:1, 129:130]        # [1,1] f32 = EPS

    # ---- mask / scalar state (ping-pong pools)
    mstate = pool("mstate", 2)
    mb_k0 = mstate.tile([128, 1], F32, tag="mbk0")
    mb_k1 = mstate.tile([69, 1], F32, tag="mbk1")
    mrow_bias = mstate.tile([1, T], F32, tag="mrowb")
    mask_row = mstate.tile([1, NPATCH], F32, tag="maskrow")
    nf = mstate.tile([1, 1], F32, tag="nf")
    pmass = mstate.tile([1, 1], F32, tag="pmass")
    pvalid = mstate.tile([1, 1], F32, tag="pvalid")
    nc.vector.memset(mb_k0[:], 0.0)
    nc.vector.memset(mb_k1[:], 0.0)
    nc.vector.memset(mrow_bias[:], 0.0)
    nc.vector.memset(mask_row[:], 1.0)
    nc.vector.memset(nf[:], float(NPATCH))
    nc.vector.memset(pmass[:], 0.0)
    nc.vector.memset(pvalid[:], 0.0)

    # ---- embed: x[:, j, item*197 + 1 + n] = patches @ wpatch + pos/bias; CLS col
    with tc.tile_pool(name="embed", bufs=1) as ep:
        wpatch = ep.tile([128, PT, D], BF16, tag="wpatch")
        nc.sync.dma_start(out=wpatch[:], in_=tens['wpatch'].ap().rearrange("j p f -> p j f"))
        xt = ep.tile([128, PT, BL * NPATCH], BF16, tag="xt")
        nc.sync.dma_start(out=xt[:], in_=tens['xt'].ap())
        pos_cls = ep.tile([128, PT, T + 1], F32, tag="poscls")
        nc.sync.dma_start(out=pos_cls[:], in_=tens['pos_cls'].ap())
        for b in range(BL):
            for fo in range(PT):
                pe = ppool['pbig'].tile([128, NPATCH], F32, tag="pbig")
                for j in range(PT):
                    nc.tensor.matmul(pe[:], wpatch[:, j, fo * 128:(fo + 1) * 128],
                                     xt[:, j, b * NPATCH:(b + 1) * NPATCH],
                                     start=(j == 0), stop=(j == PT - 1))
                nc.vector.tensor_tensor(out=x[:, fo, b * T + 1:(b + 1) * T],
                                        in0=pe[:], in1=pos_cls[:, fo, 1:197], op=OP.add)
                nc.vector.tensor_copy(out=x[:, fo, b * T:b * T + 1], in_=pos_cls[:, fo, 0:1])

    # ---- transformer layers (layer-0 weights + LN1(chunk 0) staged here; each
    # layer stages the next layer's)
    if DEPTH_BUILD > 0:
        wq_t = pool("w", 1).tile([128, PT, FF], BF16, tag="wbig1")
        nc.sync.dma_start(out=wq_t[:, :, 0:3 * D],
                          in_=tens['wqkv'].ap()[0].rearrange("j p f -> p j f"))
        wp_t = pool("w", 1).tile([128, FT, D], BF16, tag="wbig2")
        nc.sync.dma_start(out=wp_t[:, 0:PT, :],
                          in_=tens['wproj'].ap()[0].rearrange("j p f -> p j f"))
        ppool0 = {'pbig': pool("psbig", 4, "PSUM")}
        h0 = pool("h", 3).tile([128, PT, TCW], BF16, tag="h")
        _ln(nc, pool, x, 0, ones128, eps_col, h0, ppool0)
    for l in range(DEPTH_BUILD):
        (mb_k0, mb_k1, mrow_bias, mask_row, nf, pmass, pvalid,
         wq_t, wp_t, h0) = _layer(
            nc, tc, pool, tens, l, x, biases, ones_bf, ones128, consts_bf, identf, onesf,
            eps_col, eps_ap, wq_t, wp_t, h0,
            mb_k0, mb_k1, mrow_bias, mask_row, nf, pmass, pvalid)

    if dbgx_t is not None:
        nc.sync.dma_start(out=dbgx_t.ap(), in_=x[:])

    # ---- final LN on CLS tokens + head
    hp = pool("head", 1)
    xcls = hp.tile([128, PT, BL], F32, tag="xcls")
    src = bass.AP(tensor=x.tensor, offset=x[:].offset,
                  ap=[x[:].ap[0], [TOK, PT], [T, BL]])
    nc.vector.tensor_copy(out=xcls[:], in_=src)
    xcb = hp.tile([128, PT, BL], BF16, tag="xcb")
    x2b = hp.tile([128, PT, BL], BF16, tag="x2b")
    nc.vector.tensor_copy(out=xcb[:], in_=xcls[:])
    nc.vector.tensor_tensor(out=x2b[:], in0=xcls[:], in1=xcls[:], op=OP.mult)
    ps_s = ppool['ps'].tile([1, BL], F32, tag="ps")
    for j in range(PT):
        nc.tensor.matmul(ps_s[:], ones_bf[:], xcb[:, j, :], start=(j == 0), stop=(j == PT - 1))
    ps_q = ppool['ps'].tile([1, BL], F32, tag="ps")
    for j in range(PT):
        nc.tensor.matmul(ps_q[:], ones_bf[:], x2b[:, j, :], start=(j == 0), stop=(j == PT - 1))
    mu = hp.tile([1, BL], F32, tag="hmu")
    msq = hp.tile([1, BL], F32, tag="hmsq")
    nc.vector.tensor_scalar(out=mu[:], in0=ps_s[:], scalar1=1.0 / D, scalar2=None, op0=OP.mult)
    nc.vector.tensor_scalar(out=msq[:], in0=ps_q[:], scalar1=1.0 / D, scalar2=None, op0=OP.mult)
    var = hp.tile([1, BL], F32, tag="hvar")
    nc.vector.tensor_tensor(out=var[:], in0=mu[:], in1=mu[:], op=OP.mult)
    nc.vector.tensor_tensor(out=var[:], in0=msq[:], in1=var[:], op=OP.subtract)
    lnvh = hp.tile([1, BL], F32, tag="hlnv")
    nc.scalar.activation(out=lnvh[:], in_=var[:], func=AF.Ln, bias=eps_ap, scale=1.0)
    rr = hp.tile([1, BL], F32, tag="hr")
    nc.scalar.activation(out=rr[:], in_=lnvh[:], func=AF.Exp, bias=0.0, scale=-0.5)
    mu_b = hp.tile([128, BL], F32, tag="hmub")
    r_b = hp.tile([128, BL], F32, tag="hrb")
    nc.gpsimd.partition_broadcast(mu_b[:], mu[:], channels=128)
    nc.gpsimd.partition_broadcast(r_b[:], rr[:], channels=128)
    hcls = hp.tile([128, PT, BL], BF16, tag="hcls")
    for j in range(PT):
        tmp = hp.tile([128, BL], F32, tag="htmp")
        nc.vector.tensor_tensor(out=tmp[:], in0=xcls[:, j, :], in1=mu_b[:], op=OP.subtract)
        nc.vector.tensor_tensor(out=hcls[:, j, :], in0=tmp[:], in1=r_b[:], op=OP.mult)
    bhead = hp.tile([128, 8], F32, tag="bhead")
    nc.sync.dma_start(out=bhead[:], in_=tens['bhead'].ap())
    wh = None
    for fo in range(8):
        pw = min(128, NCLS - fo * 128)
        c0 = (fo // 2) * 256
        if fo % 2 == 0:
            wh = pool("qkvsb", 1).tile([128, PT, 256], BF16, tag="qkv")
            nc.sync.dma_start(
                out=wh[:, :, 0:min(256, NCLS - c0)],
                in_=tens['whead'].ap().rearrange("j p f -> p j f")[:, :, c0:min(c0 + 256, NCLS)])
        co = fo * 128 - c0
        po = ppool['ps'].tile([128, BL], F32, tag="ps")
        for j in range(PT):
            nc.tensor.matmul(po[0:pw, :], wh[:, j, co:co + pw],
                             hcls[:, j, :], start=(j == 0), stop=(j == PT - 1))
        osb = hp.tile([128, BL], F32, tag="osb")
        nc.vector.tensor_scalar(out=osb[0:pw, :], in0=po[0:pw, :],
                                scalar1=bhead[0:pw, fo:fo + 1], scalar2=None, op0=OP.add)
        nc.sync.dma_start(
            out=out_t.ap()[:, fo * 128:fo * 128 + pw].rearrange("b p -> p b"),
            in_=osb[0:pw, :])

    for pname in reversed(list(ctx_pools)):
        ctx_pools[pname].release()


def _ln(nc, pool, x, tcix, ones128, eps_col, h_out, ppool):
    """LayerNorm (no affine) of x[:, :, chunk] -> h_out bf16 [128, PT, TCW].

    Stats use an all-ones [128,128] stationary so the sums arrive already
    broadcast across partitions (no gpsimd broadcast, no single-partition row
    chain). rsqrt(var+eps) = exp(-0.5*ln(var+eps)) on ACT: ln/exp/square share
    one activation table with softmax's exp, so no ACT table reloads."""
    sl = slice(tcix * TCW, (tcix + 1) * TCW)
    lc = pool("lncast", 3)
    lb = pool("lnb", 2)
    lt = pool("lntmp", 2)
    ps_s = ppool['pbig'].tile([128, TCW], F32, tag="pbig")
    ps_q = ppool['pbig'].tile([128, TCW], F32, tag="pbig")
    for j in range(PT):
        xb = lc.tile([128, TCW], BF16, tag="lncast")
        x2 = lc.tile([128, TCW], BF16, tag="lncast")
        nc.vector.tensor_copy(out=xb[:], in_=x[:, j, sl])
        nc.scalar.activation(out=x2[:], in_=x[:, j, sl], func=AF.Square, bias=0.0, scale=1.0)
        nc.tensor.matmul(ps_s[:], ones128, xb[:], start=(j == 0), stop=(j == PT - 1))
        nc.tensor.matmul(ps_q[:], ones128, x2[:], start=(j == 0), stop=(j == PT - 1))
    mu_b = lb.tile([128, TCW], F32, tag="lnb")
    nc.vector.tensor_scalar(out=mu_b[:], in0=ps_s[:], scalar1=1.0 / D, scalar2=None, op0=OP.mult)
    t = lt.tile([128, TCW], F32, tag="lntmp")
    nc.vector.tensor_tensor(out=t[:], in0=mu_b[:], in1=mu_b[:], op=OP.mult)
    var = lt.tile([128, TCW], F32, tag="lntmp")
    nc.vector.scalar_tensor_tensor(out=var[:], in0=ps_q[:], scalar=1.0 / D, in1=t[:],
                                   op0=OP.mult, op1=OP.subtract)
    lnv = lt.tile([128, TCW], F32, tag="lntmp")
    nc.scalar.activation(out=lnv[:], in_=var[:], func=AF.Ln, bias=eps_col, scale=1.0)
    rr_b = lb.tile([128, TCW], F32, tag="lnb")
    nc.scalar.activation(out=rr_b[:], in_=lnv[:], func=AF.Exp, bias=0.0, scale=-0.5)
    for j in range(PT):
        tmp = lt.tile([128, TCW], F32, tag="lntmp")
        nc.vector.tensor_tensor(out=tmp[:], in0=x[:, j, sl], in1=mu_b[:], op=OP.subtract)
        nc.vector.tensor_tensor(out=h_out[:, j, :], in0=tmp[:], in1=rr_b[:], op=OP.mult)


def _layer(nc, tc, pool, tens, l, x, biases, ones_bf, ones128, consts_bf, identf, onesf,
           eps_col, eps_ap, wqkv_t, wproj_t, h0,
           mb_k0, mb_k1, mrow_bias, mask_row, nf, pmass, pvalid):
    """One transformer layer. Weights for THIS layer (wqkv_t/wproj_t) and the
    LN1 of chunk 0 (h0) were staged by the previous layer so the PE never
    waits at the layer boundary. Returns (mask-state..., wqkv_next, wproj_next,
    h0_next)."""
    last = l == DEPTH_BUILD - 1
    ppool = {'pbig': pool("psbig", 4, "PSUM"), 'ps': pool("psattn", 3, "PSUM"),
             'prs': pool("psrs", 1, "PSUM")}
    wpool = pool("w", 1)
    dpool = pool("dram", 2, "DRAM")
    mp_ = pool("mask", 2)
    mtr = pool("mtrow", 3)
    ap1 = pool("attnbig", 1)
    php = pool("pth", 1)
    asm = pool("attnsm", 1)
    rbp = pool("rbp", 1)
    hp_ = pool("h", 3)
    h2p = hp_
    b_qkv = lambda fo: biases[:, l, fo:fo + 1]
    b_proj = lambda fo: biases[:, l, 18 + fo:18 + fo + 1]
    b_fc1 = lambda fo: biases[:, l, 24 + fo:24 + fo + 1]
    b_fc2 = lambda fo: biases[:, l, 48 + fo:48 + fo + 1]
    ident128 = consts_bf[:, 80:208]
    rsel = lambda hh: consts_bf[:, 208 + 12 * hh:208 + 12 * hh + 12]
    vsel = lambda j: consts_bf[:, 352 + 12 * j:352 + 12 * j + 12]
    wqkv = wqkv_t[:, :, 0:3 * D]
    wproj = wproj_t[:, 0:PT, :]

    mrow_b12 = asm.tile([12, T], F32, tag="mrowb12")
    nc.gpsimd.partition_broadcast(mrow_b12[:], mrow_bias[:], channels=12)
    imp12 = asm.tile([12, T], F32, tag="imp12")
    first_imp = [True]

    h_tiles = [None] * NTC
    h_tiles[0] = h0
    h2_tiles = [None] * NTC
    qkv_tiles = [None] * NTC
    wfc_t = [None, None]

    def _qkv_gemm(tcix):
        qkv = pool("qkvsb", 1).tile([128, QT, TCW], BF16, tag="qkv")
        qkv_tiles[tcix] = qkv
        h = h_tiles[tcix]
        # v-tiles (fo 12..17) first so attention's transposes can start early
        for fo in list(range(12, QT)) + list(range(6, 12)) + list(range(6)):
            pq = ppool['pbig'].tile([128, TCW], F32, tag="pbig")
            for j in range(PT):
                nc.tensor.matmul(pq[:], wqkv[:, j, fo * 128:(fo + 1) * 128], h[:, j, :],
                                 start=(j == 0), stop=(j == PT - 1))
            nc.scalar.activation(out=qkv[:, fo, :], in_=pq[:], func=AF.Identity,
                                 bias=b_qkv(fo), scale=1.0)

    def _attn(tcix):
        qkv = qkv_tiles[tcix]
        av_t = pool("gav", 1).tile([128, FT, TCW], BF16, tag="gav")
        av = av_t[:, 0:PT, :]

        # ---- vT per ptile per item (PE transposes; psum->sbuf copies split
        # across ACT and DVE)
        vt0 = ap1.tile([128, 2, D], BF16, tag="vt0")
        vt1 = ap1.tile([69, 2, D], BF16, tag="vt1")
        for b in range(2):
            for j in range(PT):
                tp0 = ppool['ps'].tile([128, 128], BF16, tag="ps")
                nc.tensor.transpose(tp0[:], qkv[:, 12 + j, b * T:b * T + 128], ident128)
                nc.scalar.activation(out=vt0[:, b, j * 128:(j + 1) * 128], in_=tp0[:],
                                     func=AF.Copy, bias=0.0, scale=1.0)
                tp1 = ppool['ps'].tile([69, 128], BF16, tag="ps")
                nc.tensor.transpose(tp1[:], qkv[:, 12 + j, b * T + 128:(b + 1) * T], ident128)
                nc.vector.tensor_copy(out=vt1[:, b, j * 128:(j + 1) * 128], in_=tp1[:])

        # ---- vnorm rows, all heads: [12, TCW]; sqrt(p) = exp(0.5*ln(p+eps))
        pvn = ppool['ps'].tile([12, TCW], F32, tag="ps")
        for j in range(PT):
            v2 = pool("lncast", 3).tile([128, TCW], BF16, tag="lncast")
            nc.vector.tensor_tensor(out=v2[:], in0=qkv[:, 12 + j, :], in1=qkv[:, 12 + j, :],
                                    op=OP.mult)
            nc.tensor.matmul(pvn[:], vsel(j), v2[:], start=(j == 0), stop=(j == PT - 1))
        lnvn = pool("lntmp", 2).tile([12, TCW], F32, tag="lntmp")
        nc.scalar.activation(out=lnvn[:], in_=pvn[:], func=AF.Ln, bias=eps_col[0:12, :],
                             scale=1.0)
        vnr = asm.tile([12, TCW], F32, tag="vnr")
        nc.scalar.activation(out=vnr[:], in_=lnvn[:], func=AF.Exp, bias=0.0, scale=0.5)

        # ---- CLS attention rows, all heads: q_cls one-hot stationary + 12 MMs
        qcb = asm.tile([128, PT, 24], BF16, tag="qcb")
        nc.vector.memset(qcb[:], 0.0)
        for b in range(2):
            for j in range(PT):
                c = b * 12 + 2 * j
                nc.vector.tensor_copy(out=qcb[0:64, j, c:c + 1],
                                      in_=qkv[0:64, j, b * T:b * T + 1])
                nc.vector.tensor_copy(out=qcb[64:128, j, c + 1:c + 2],
                                      in_=qkv[64:128, j, b * T:b * T + 1])
        pcls = ppool['ps'].tile([12, TCW], F32, tag="ps")
        for b in range(2):
            for j in range(PT):
                nc.tensor.matmul(pcls[:, b * T:(b + 1) * T], qcb[:, j, b * 12:b * 12 + 12],
                                 qkv[:, 6 + j, b * T:(b + 1) * T],
                                 start=(j == 0), stop=(j == PT - 1))
        for b in range(2):
            crow = asm.tile([12, T], F32, tag="crow")
            nc.vector.tensor_tensor(out=crow[:], in0=pcls[:, b * T:(b + 1) * T],
                                    in1=mrow_b12[:], op=OP.add)
            erow = asm.tile([12, T], F32, tag="erow")
            esum = asm.tile([12, 1], F32, tag="esum")
            nc.scalar.activation(out=erow[:], in_=crow[:], func=AF.Exp, bias=0.0, scale=1.0,
                                 accum_out=esum[:])
            ercp = asm.tile([12, 1], F32, tag="ercp")
            nc.vector.reciprocal(out=ercp[:], in_=esum[:])
            contrib = asm.tile([12, T], F32, tag="contrib")
            nc.vector.scalar_tensor_tensor(out=contrib[:], in0=erow[:], scalar=ercp[:],
                                           in1=vnr[:, b * T:(b + 1) * T],
                                           op0=OP.mult, op1=OP.mult)
            if first_imp[0]:
                nc.vector.tensor_copy(out=imp12[:], in_=contrib[:])
                first_imp[0] = False
            else:
                nc.vector.tensor_tensor(out=imp12[:], in0=imp12[:], in1=contrib[:], op=OP.add)

        # ---- scores / exp / row-sums / av, in two 6-head half-groups: av of
        # heads 0-5 overlaps scoring of heads 6-11, and pt buffers are halved
        # rsel6(hh): [128, 6] one-hot column hh%6 (sub-slice of the rsel block)
        rsel6 = lambda hh: consts_bf[:, 208 + 12 * hh + 6 * (hh // 6):
                                     208 + 12 * hh + 6 * (hh // 6) + 6]
        for hg in range(2):
            pt0 = php.tile([128, 6, TCW], BF16, tag="pt0")
            pt1 = php.tile([69, 6, TCW], BF16, tag="pt1")
            prs_h = ppool['prs'].tile([6, TCW], F32, tag="prs")

            def _scores(hh):
                hl = hh % 6
                bsl = slice((hh % 2) * 64, (hh % 2) * 64 + 64)
                jq = hh // 2
                psc0 = ppool['ps'].tile([128, TCW], F32, tag="ps")
                psc1 = ppool['ps'].tile([69, TCW], F32, tag="ps")
                for b in range(2):
                    q_ap = qkv[bsl, jq, b * T:(b + 1) * T]
                    nc.tensor.matmul(psc0[:, b * T:(b + 1) * T],
                                     qkv[bsl, 6 + jq, b * T:b * T + 128], q_ap,
                                     start=True, stop=True)
                    nc.tensor.matmul(psc1[:, b * T:(b + 1) * T],
                                     qkv[bsl, 6 + jq, b * T + 128:(b + 1) * T], q_ap,
                                     start=True, stop=True)
                nc.scalar.activation(out=pt0[:, hl, :], in_=psc0[:], func=AF.Exp,
                                     bias=mb_k0[:], scale=1.0)
                nc.scalar.activation(out=pt1[:, hl, :], in_=psc1[:], func=AF.Exp,
                                     bias=mb_k1[0:69, :], scale=1.0)

            def _rowsum(hh):
                hl = hh % 6
                nc.tensor.matmul(prs_h[:], rsel6(hh), pt0[:, hl, :],
                                 start=(hl == 0), stop=False)
                nc.tensor.matmul(prs_h[:], rsel6(hh)[0:69, :], pt1[:, hl, :],
                                 start=False, stop=(hl == 5))

            for hh in range(hg * 6, hg * 6 + 6):
                _scores(hh)
                if hh % 6 >= 1:
                    _rowsum(hh - 1)
            _rowsum(hg * 6 + 5)

            # softmax normalize + av for this half's 3 ptiles; the per-head
            # 1/rowsum rows are replicated across their 64 feature partitions
            # by a tiny PE matmul against block-select constants
            rrow6f = pool("lntmp", 2).tile([6, TCW], F32, tag="lntmp")
            nc.vector.reciprocal_approx_fast(out=rrow6f[:], in_=prs_h[:])
            rrow6 = pool("lncast", 3).tile([6, TCW], BF16, tag="lncast")
            nc.vector.tensor_copy(out=rrow6[:], in_=rrow6f[:])
            for p in range(hg * 3, hg * 3 + 3):
                pl = p - hg * 3
                rbps = ppool['ps'].tile([128, TCW], F32, tag="ps")
                nc.tensor.matmul(rbps[:], consts_bf[0:6, 424 + pl * 128:424 + (pl + 1) * 128],
                                 rrow6[:], start=True, stop=True)
                rb = rbp.tile([128, TCW], F32, tag="rb")
                nc.vector.tensor_copy(out=rb[:], in_=rbps[:])
                pav = ppool['ps'].tile([128, TCW], F32, tag="ps")
                for b in range(2):
                    tsl = slice(b * T, (b + 1) * T)
                    for hi in range(2):
                        hh = 2 * p + hi
                        hl = hh % 6
                        osl = slice(hi * 64, hi * 64 + 64)
                        nc.tensor.matmul(pav[osl, tsl], vt0[:, b, hh * 64:(hh + 1) * 64],
                                         pt0[:, hl, tsl], start=True, stop=False)
                        nc.tensor.matmul(pav[osl, tsl], vt1[:, b, hh * 64:(hh + 1) * 64],
                                         pt1[:, hl, tsl], start=False, stop=True)
                nc.vector.tensor_tensor(out=av[:, p, :], in0=pav[:], in1=rb[:], op=OP.mult)

        # ---- proj + residual
        for fo in range(PT):
            pp = ppool['pbig'].tile([128, TCW], F32, tag="pbig")
            for j in range(PT):
                nc.tensor.matmul(pp[:], wproj[:, j, fo * 128:(fo + 1) * 128], av[:, j, :],
                                 start=(j == 0), stop=(j == PT - 1))
            sl = slice(tcix * TCW, (tcix + 1) * TCW)
            nc.vector.scalar_tensor_tensor(out=x[:, fo, sl], in0=pp[:], scalar=b_proj(fo),
                                           in1=x[:, fo, sl], op0=OP.add, op1=OP.add)

    # ---- attention-half driver: LN(c+1) is emitted right after qkv(c) so its
    # serial chain hides under attention(c)'s PE work; LN2(c) follows proj(c)
    # and hides under attention(c+1); this-layer's wfc1/wfc2 DMAs start as
    # soon as their (time-shared) buffers' last readers are emitted.
    for tcix in range(NTC):
        _qkv_gemm(tcix)
        if tcix + 1 < NTC:
            h_next = hp_.tile([128, PT, TCW], BF16, tag="h")
            h_tiles[tcix + 1] = h_next
            _ln(nc, pool, x, tcix + 1, ones128, eps_col, h_next, ppool)
        if tcix == NTC - 1:
            wfc1_t = wpool.tile([128, PT, FF], BF16, tag="wbig1")
            wfc_t[0] = wfc1_t
            nc.sync.dma_start(out=wfc1_t[:],
                              in_=tens['wfc1'].ap()[l].rearrange("j p f -> p j f"))
        _attn(tcix)
        if tcix == NTC - 1:
            wfc2_t = wpool.tile([128, FT, D], BF16, tag="wbig2")
            wfc_t[1] = wfc2_t
            nc.sync.dma_start(out=wfc2_t[:],
                              in_=tens['wfc2'].ap()[l].rearrange("j p f -> p j f"))
        if tcix == 0:
            h2_new = h2p.tile([128, PT, TCW], BF16, tag="h")
            h2_tiles[0] = h2_new
            _ln(nc, pool, x, 0, ones128, eps_col, h2_new, ppool)

    # ---- importance: reduce heads, scale, start AllReduce (result consumed
    # after the MLP so the collective latency hides under GEMMs)
    pimp = ppool['ps'].tile([1, T], F32, tag="ps")
    nc.tensor.matmul(pimp[:], onesf[0:12, :], imp12[:], start=True, stop=True)
    impw = mtr.tile([1, NPATCH], F32, tag="mtrow")
    nc.vector.tensor_scalar(out=impw[:], in0=pimp[0:1, 1:T], scalar1=1.0 / (B * H),
                            scalar2=None, op0=OP.mult)
    in_b = dpool.tile([1, NPATCH], F32, tag="ccin")
    out_b = dpool.tile([1, NPATCH], F32, tag="ccout")
    nc.gpsimd.dma_start(out=in_b[:], in_=impw[:])
    nc.gpsimd.collective_compute(
        "AllReduce", OP.add, replica_groups=[list(range(NCORE))],
        ins=[in_b[:].opt()], outs=[out_b[:].opt()])

    # ---- MLP (wfc1/wfc2 were DMA'd during the attention half)
    wfc1, wfc2 = wfc_t

    def _mask_chain():
        imp_g = mp_.tile([1, NPATCH], F32, tag="impg")
        nc.gpsimd.dma_start(out=imp_g[:], in_=out_b[:])

        mass = mp_.tile([1, 1], F32, tag="mass")
        nc.vector.tensor_reduce(out=mass[:], in_=imp_g[:], axis=AX, op=OP.add)
        me = mp_.tile([1, 1], F32, tag="me")
        nc.vector.tensor_scalar(out=me[:], in0=mass[:], scalar1=EPS, scalar2=None, op0=OP.add)
        mrec = mp_.tile([1, 1], F32, tag="mrec")
        nc.vector.reciprocal(out=mrec[:], in_=me[:])
        p_r = mtr.tile([1, NPATCH], F32, tag="mtrow")
        nc.vector.tensor_scalar(out=p_r[:], in0=imp_g[:], scalar1=mrec[:], scalar2=None, op0=OP.mult)
        lnp = mtr.tile([1, NPATCH], F32, tag="mtrow")
        nc.scalar.activation(out=lnp[:], in_=p_r[:], func=AF.Ln, bias=eps_ap, scale=1.0)
        pl = mtr.tile([1, NPATCH], F32, tag="mtrow")
        nc.vector.tensor_tensor(out=pl[:], in0=p_r[:], in1=lnp[:], op=OP.mult)
        entn = mp_.tile([1, 1], F32, tag="entn")     # = -entropy
        nc.vector.tensor_reduce(out=entn[:], in_=pl[:], axis=AX, op=OP.add)
        lnN = mp_.tile([1, 1], F32, tag="lnN")
        nc.scalar.activation(out=lnN[:], in_=nf[:], func=AF.Ln, bias=0.0, scale=1.0)
        lrec = mp_.tile([1, 1], F32, tag="lrec")
        nc.vector.reciprocal(out=lrec[:], in_=lnN[:])
        nrho = mp_.tile([1, 1], F32, tag="nrho")     # = -rho
        nc.vector.tensor_tensor(out=nrho[:], in0=entn[:], in1=lrec[:], op=OP.mult)
        pme = mp_.tile([1, 1], F32, tag="pme")
        nc.vector.tensor_scalar(out=pme[:], in0=pmass[:], scalar1=EPS, scalar2=None, op0=OP.add)
        pmr = mp_.tile([1, 1], F32, tag="pmr")
        nc.vector.reciprocal(out=pmr[:], in_=pme[:])
        ratio = mp_.tile([1, 1], F32, tag="ratio")
        nc.vector.tensor_tensor(out=ratio[:], in0=mass[:], in1=pmr[:], op=OP.mult)
        nrr = mp_.tile([1, 1], F32, tag="nrr")
        nc.vector.tensor_tensor(out=nrr[:], in0=nrho[:], in1=ratio[:], op=OP.mult)
        kr = mp_.tile([1, 1], F32, tag="kr")
        nc.vector.tensor_scalar(out=kr[:], in0=nrr[:], scalar1=GAMMA, scalar2=1.0,
                                op0=OP.mult, op1=OP.add)
        nc.vector.tensor_scalar(out=kr[:], in0=kr[:], scalar1=1.0, scalar2=0.0,
                                op0=OP.min, op1=OP.max)
        pvc = mp_.tile([1, 1], F32, tag="pvc")
        nc.vector.tensor_scalar(out=pvc[:], in0=pvalid[:], scalar1=-1.0, scalar2=1.0,
                                op0=OP.mult, op1=OP.add)
        krm = mp_.tile([1, 1], F32, tag="krm")
        nc.vector.scalar_tensor_tensor(out=krm[:], in0=kr[:], scalar=pvalid[:], in1=pvc[:],
                                       op0=OP.mult, op1=OP.add)
        y = mp_.tile([1, 1], F32, tag="y")
        nc.vector.tensor_tensor(out=y[:], in0=nf[:], in1=krm[:], op=OP.mult)
        m_th = mp_.tile([1, 1], F32, tag="m_th")
        nc.vector.tensor_tensor(out=m_th[:], in0=y[:], in1=nf[:], op=OP.min)
        nc.vector.tensor_scalar(out=m_th[:], in0=m_th[:], scalar1=-1.0, scalar2=None, op0=OP.add)

        # ranks: imp_r = imp_g + (mask-1)*1e30 ; rank[p] = #{f: imp_r[f] > imp_r[p]}
        pen = mtr.tile([1, NPATCH], F32, tag="mtrow")
        nc.vector.tensor_scalar(out=pen[:], in0=mask_row[:], scalar1=1e30, scalar2=-1e30,
                                op0=OP.mult, op1=OP.add)
        imp_r = mp_.tile([1, NPATCH], F32, tag="impr")
        nc.vector.tensor_tensor(out=imp_r[:], in0=imp_g[:], in1=pen[:], op=OP.add)
        # columns via K=1 f32 matmuls
        c0p = ppool['ps'].tile([128, 1], F32, tag="ps")
        nc.tensor.matmul(c0p[:], imp_r[0:1, 0:128], onesf[0:1, :], start=True, stop=True)
        c1p = ppool['ps'].tile([68, 1], F32, tag="ps")
        nc.tensor.matmul(c1p[:], imp_r[0:1, 128:NPATCH], onesf[0:1, :], start=True, stop=True)
        c0 = mp_.tile([128, 1], F32, tag="c0")
        c1 = mp_.tile([68, 1], F32, tag="c1")
        nc.vector.tensor_copy(out=c0[:], in_=c0p[:])
        nc.vector.tensor_copy(out=c1[:], in_=c1p[:])
        ib0 = pool("mbig", 1).tile([128, NPATCH], F32, tag="ib0")
        nc.gpsimd.partition_broadcast(ib0[:], imp_r[:], channels=128)
        junk = pool("mbig", 1).tile([128, NPATCH], F32, tag="junkr")
        rk0 = mp_.tile([128, 1], F32, tag="rk0")
        nc.vector.tensor_scalar(out=junk[:], in0=ib0[:], scalar1=c0[:], scalar2=0.0,
                                op0=OP.is_gt, op1=OP.add, accum_out=rk0[:])
        rk1 = mp_.tile([68, 1], F32, tag="rk1")
        nc.vector.tensor_scalar(out=junk[0:68, :], in0=ib0[0:68, :], scalar1=c1[:], scalar2=0.0,
                                op0=OP.is_gt, op1=OP.add, accum_out=rk1[:])
        mthb = mp_.tile([128, 1], F32, tag="mthb")
        nc.gpsimd.partition_broadcast(mthb[:], m_th[:], channels=128)
        ca0 = mp_.tile([128, 1], F32, tag="ca0")
        nc.vector.tensor_scalar(out=ca0[:], in0=rk0[:], scalar1=15.5, scalar2=None, op0=OP.is_lt)
        cb0 = mp_.tile([128, 1], F32, tag="cb0")
        nc.vector.tensor_scalar(out=cb0[:], in0=rk0[:], scalar1=mthb[:], scalar2=None, op0=OP.is_le)
        nmask0 = mp_.tile([128, 1], F32, tag="nmask0")
        nc.vector.tensor_tensor(out=nmask0[:], in0=ca0[:], in1=cb0[:], op=OP.max)
        ca1 = mp_.tile([68, 1], F32, tag="ca1")
        nc.vector.tensor_scalar(out=ca1[:], in0=rk1[:], scalar1=15.5, scalar2=None, op0=OP.is_lt)
        cb1 = mp_.tile([68, 1], F32, tag="cb1")
        nc.vector.tensor_scalar(out=cb1[:], in0=rk1[:], scalar1=mthb[0:68, :], scalar2=None,
                                op0=OP.is_le)
        nmask1 = mp_.tile([68, 1], F32, tag="nmask1")
        nc.vector.tensor_tensor(out=nmask1[:], in0=ca1[:], in1=cb1[:], op=OP.max)

        # mask columns -> row via PE transpose (f32)
        mr0p = ppool['ps'].tile([1, 128], F32, tag="ps")
        nc.tensor.transpose(mr0p[:], nmask0[:], identf)
        mr1p = ppool['ps'].tile([1, 68], F32, tag="ps")
        nc.tensor.transpose(mr1p[:], nmask1[:], identf[0:68, 0:68])
        mask_row_n = mp_.tile([1, NPATCH], F32, tag="maskrow")
        nc.vector.tensor_copy(out=mask_row_n[0:1, 0:128], in_=mr0p[:])
        nc.vector.tensor_copy(out=mask_row_n[0:1, 128:NPATCH], in_=mr1p[:])

        # key-space additive bias state for next layer (CLS row = 0 bias)
        mkb = dpool.tile([1, T], F32, tag="mkb")
        nc.gpsimd.dma_start(out=mkb[0:1, 0:1], in_=onesf[0:1, :])
        nc.gpsimd.dma_start(out=mkb[0:1, 1:T], in_=mask_row_n[:])
        mb_k0_n = mp_.tile([128, 1], F32, tag="mbk0")
        mb_k1_n = mp_.tile([69, 1], F32, tag="mbk1")
        nc.gpsimd.dma_start(out=mb_k0_n[:], in_=mkb[0, 0:128].unsqueeze(1))
        nc.gpsimd.dma_start(out=mb_k1_n[:], in_=mkb[0, 128:T].unsqueeze(1))
        nc.vector.tensor_scalar(out=mb_k0_n[:], in0=mb_k0_n[:], scalar1=MB, scalar2=-MB,
                                op0=OP.mult, op1=OP.add)
        nc.vector.tensor_scalar(out=mb_k1_n[:], in0=mb_k1_n[:], scalar1=MB, scalar2=-MB,
                                op0=OP.mult, op1=OP.add)
        mrow_bias_n = mp_.tile([1, T], F32, tag="mrowb")
        nc.vector.memset(mrow_bias_n[0:1, 0:1], 0.0)
        nc.vector.tensor_scalar(out=mrow_bias_n[0:1, 1:T], in0=mask_row_n[:], scalar1=MB,
                                scalar2=-MB, op0=OP.mult, op1=OP.add)

        nf_n = mp_.tile([1, 1], F32, tag="nf")
        nc.vector.tensor_reduce(out=nf_n[:], in_=mask_row_n[:], axis=AX, op=OP.add)
        pvalid_n = mp_.tile([1, 1], F32, tag="pvalid")
        nc.vector.tensor_scalar(out=pvalid_n[:], in0=nf[:], scalar1=16.5, scalar2=None, op0=OP.is_gt)
        pmass_n = mp_.tile([1, 1], F32, tag="pmass")
        nc.vector.tensor_copy(out=pmass_n[:], in_=mass[:])
        return mb_k0_n, mb_k1_n, mrow_bias_n, mask_row_n, nf_n, pmass_n, pvalid_n

    mask_res = [None]
    wqkv_next = wproj_next = h0_next = None
    for tcix in range(NTC):
        h2 = h2_tiles[tcix]
        g = pool("gav", 1).tile([128, FT, TCW], BF16, tag="gav")
        for fo in range(FT):
            pf = ppool['pbig'].tile([128, TCW], F32, tag="pbig")
            for j in range(PT):
                nc.tensor.matmul(pf[:], wfc1[:, j, fo * 128:(fo + 1) * 128], h2[:, j, :],
                                 start=(j == 0), stop=(j == PT - 1))
            nc.scalar.activation(out=g[:, fo, :], in_=pf[:], func=AF.Gelu,
                                 bias=b_fc1(fo), scale=1.0)
        if tcix + 1 < NTC:
            h2_new = h2p.tile([128, PT, TCW], BF16, tag="h")
            h2_tiles[tcix + 1] = h2_new
            _ln(nc, pool, x, tcix + 1, ones128, eps_col, h2_new, ppool)
        if tcix == NTC - 1:
            # mask chain + next-layer staging: all hide under chunk 3's fc2
            # GEMMs. ACT-table order stays [... gelu | mask ln | next-layer
            # ln/exp ...] -> exactly two table loads per layer.
            mask_res[0] = _mask_chain()
            if not last:
                wqkv_next = wpool.tile([128, PT, FF], BF16, tag="wbig1")
                nc.sync.dma_start(out=wqkv_next[:, :, 0:3 * D],
                                  in_=tens['wqkv'].ap()[l + 1].rearrange("j p f -> p j f"))
                h0_next = hp_.tile([128, PT, TCW], BF16, tag="h")
                _ln(nc, pool, x, 0, ones128, eps_col, h0_next, ppool)
        sl = slice(tcix * TCW, (tcix + 1) * TCW)
        for fo in range(PT):
            pf2 = ppool['pbig'].tile([128, TCW], F32, tag="pbig")
            for k in range(FT):
                nc.tensor.matmul(pf2[:], wfc2[:, k, fo * 128:(fo + 1) * 128], g[:, k, :],
                                 start=(k == 0), stop=(k == FT - 1))
            nc.vector.scalar_tensor_tensor(out=x[:, fo, sl], in0=pf2[:], scalar=b_fc2(fo),
                                           in1=x[:, fo, sl], op0=OP.add, op1=OP.add)
    if not last:
        wproj_next = wpool.tile([128, FT, D], BF16, tag="wbig2")
        nc.sync.dma_start(out=wproj_next[:, 0:PT, :],
                          in_=tens['wproj'].ap()[l + 1].rearrange("j p f -> p j f"))
    return mask_res[0] + (wqkv_next, wproj_next, h0_next)


# ---------------------------------------------------------------- entry point
_CACHE = {}


def _get_nc():
    key = (DEPTH_BUILD, bool(os.environ.get('KERNEL_DEBUG_X')))
    if key not in _CACHE:
        nc = bacc.Bacc("TRN2", target_bir_lowering=False, debug=False, num_devices=NCORE)
        dbg = build(nc)
        nc.compile()
        _CACHE[key] = (nc, dbg)
    return _CACHE[key]


def kernel(**inputs):
    nc, dbg = _get_nc()
    shared = prep_weights(inputs)
    in_maps = []
    for c in range(NCORE):
        m = dict(shared)
        m['xt'] = prep_x_shard(inputs['x'], c)
        in_maps.append(m)
    res = run_bass_kernel_spmd(nc, in_maps, core_ids=list(range(NCORE)),
                               trace=bool(os.environ.get('KERNEL_TRACE')))
    kernel.last_results = res
    out = np.concatenate([res.results[c]['out'] for c in range(NCORE)], axis=0)
    return out.astype(np.float32)

